# revision 18
# baseline (speedup 1.0000x reference)
"""Trainium2 Bass kernel for the pairwise-MLP adjacency module.

Computes out[b,i,j] = softmax_j( MLP(|v[b,i,:] - v[b,j,:]|) ) where the MLP is
128 -> 64 (leaky 0.1) -> 32 (leaky 0.1) -> 1, implemented as 1x1 convs in the
reference.

Sharding: 8 cores, 2 cores per batch element b (B=4); each core computes 256
of the 512 softmax rows for its b. Weights are replicated, packed host-side.

Per-core dataflow (v3, software-pipelined):
  - DVE: phi_i = relu(VT - v_i) as one fused tensor_scalar (sub + max0).
  - PE L1: z1 = 2*W1@relu(d) - W1@x + (W1@v_i + b1); two rows share a
    (128,512) PSUM tile (partition halves), the -W1@x term is one shared
    M=128 matmul (m1t), and the per-row constant rides the Prelu bias.
  - ACT: leaky-relu + bias fused into every PSUM->SBUF move (Prelu, 0.1).
  - PE L2: block-diag2(W2T) processes 2 rows per matmul; two consecutive
    steps write the two (128,512) halves of a (128,1024) 2-bank PSUM tile.
  - ACT: ONE Prelu evacuates each (128,1024) h2 pair (b2 bias is constant
    across rows, so pairing is legal; amortizes the ScalarE fixed cost).
  - PE L3: 8 shifted zero-padded copies of block-diag4(W3T) accumulate logits
    for 128 rows densely into one PSUM bank (4 col-groups x 8 shifts).
  - ACT: softmax via single Exp with fused row-sum (accum_out); DVE
    reciprocal + scale.  b3 dropped (softmax shift-invariant).
  - The PE stream at step e is [6x L1(e+2), 2x L2(e), L3(e-2)] so every
    matmul's inputs are ready well before issue (minimizes PE stalls, which
    otherwise hold the tensor engine in its low-clock pstate).
"""

import sys

for _p in ("/opt/trn_rl_repo",):
    if _p not in sys.path:
        sys.path.insert(0, _p)

from contextlib import ExitStack

import numpy as np
import ml_dtypes

import concourse.bass as bass
import concourse.bacc as bacc
import concourse.tile as tile
from concourse import mybir
from concourse.bass_utils import run_bass_kernel_spmd

BF16 = ml_dtypes.bfloat16


def _register_prelu_bias_op():
    """Register a custom DVE op computing Prelu(in0 + s0) with slope imm2,
    used to offload part of the PSUM->SBUF h1 evacuation from the (busy)
    scalar engine to the vector engine.  Uses the documented custom-DVE
    extension point (concourse.dve_ops.OPS); the uops_sha pin is computed
    from the same lower() call DveOp.compile uses."""
    import numpy as np
    from concourse import dve_ops as dvo
    from concourse.dve_spec import Spec, Src0, C0, C2, maxx
    from concourse.dve_spec import lower as dve_lower
    from concourse.dve_uop import DveOpSpec

    name = "PRELU_BIAS_ANT"
    for op in dvo.OPS:
        if op.name == name:
            return op

    def _ref(in0, in1, c0, c1, c2):
        u = np.asarray(in0, np.float32) + c0
        return np.maximum(u, u * c2)

    u = Src0 + C0
    spec = Spec(body=maxx(u, u * C2), reference=_ref)
    op = dvo.DveOp(name, spec, subdim=False, uops_sha={})
    dvo.OPS.append(op)
    dvo.CUSTOM_DVE_SPECS[name] = spec
    dvo._SUB_OPCODE_FOR_NAME[name] = dvo._CUSTOM_DVE_ROW_BASE + len(dvo.OPS) - 1
    for ver in ("v3", "v4"):
        r = DveOpSpec(
            name=name,
            opcode=dvo.get_dve_sub_opcode(name),
            uops=dve_lower(spec, ver=ver),
            rd1_en=dvo.has_src1(spec),
        )
        op.uops_sha[ver] = r.sha(ver)
    return op


PRELU_BIAS = _register_prelu_bias_op()

B, N, D = 4, 512, 128
H1, H2 = 64, 32
SLOPE = 0.1
NCORES = 8
IPC = B * N // NCORES      # 256 rows per core
NQ = IPC // 4              # 64 steps of 4 rows
QPB = NQ // 2              # 32 steps per softmax batch of 128 rows


def build_nc():
    f32 = mybir.dt.float32
    bf = mybir.dt.bfloat16
    nc = bacc.Bacc("TRN2", target_bir_lowering=False, debug=False)

    vt = nc.dram_tensor("vt", [D, N], bf, kind="ExternalInput").ap()
    vtq = nc.dram_tensor("vtq", [D, IPC], f32, kind="ExternalInput").ap()
    w1t2 = nc.dram_tensor("w1t2", [D, H1], bf, kind="ExternalInput").ap()
    m1t = nc.dram_tensor("m1t", [D, 128], bf, kind="ExternalInput").ap()
    w2bd = nc.dram_tensor("w2bd", [2 * H1, 2 * H2], bf, kind="ExternalInput").ap()
    w3v = nc.dram_tensor("w3v", [128, 8 * H2], bf, kind="ExternalInput").ap()
    cbias = nc.dram_tensor("cbias", [128, IPC // 2], f32, kind="ExternalInput").ap()
    b2s = nc.dram_tensor("b2s", [128, 1], f32, kind="ExternalInput").ap()
    outd = nc.dram_tensor("out", [IPC, N], f32, kind="ExternalOutput").ap()

    LR = mybir.ActivationFunctionType.Prelu  # parametric relu: reads alpha
    EXP = mybir.ActivationFunctionType.Exp
    SUB = mybir.AluOpType.subtract
    MAX = mybir.AluOpType.max

    with tile.TileContext(nc) as tc, ExitStack() as ctx:
        singles = ctx.enter_context(tc.tile_pool(name="singles", bufs=1))
        phip = ctx.enter_context(tc.tile_pool(name="phip", bufs=16))
        h1p = ctx.enter_context(tc.tile_pool(name="h1p", bufs=6))
        h2p = ctx.enter_context(tc.tile_pool(name="h2p", bufs=2))
        p1p = ctx.enter_context(tc.tile_pool(name="p1p", bufs=4, space="PSUM"))
        p2p = ctx.enter_context(tc.tile_pool(name="p2p", bufs=1, space="PSUM"))
        lgp = ctx.enter_context(tc.tile_pool(name="lgp", bufs=2, space="PSUM"))
        postp = ctx.enter_context(tc.tile_pool(name="postp", bufs=2))

        # dummy 1-col Prelu: pulls the ACT table load into the prologue where
        # it overlaps the input DMAs instead of stalling the first h1 act.
        warm0 = singles.tile([128, 1], f32)
        nc.vector.memset(warm0, 0.0)
        warm = singles.tile([128, 1], f32)
        nc.scalar.activation(
            out=warm, in_=warm0, func=LR, scale=1.0, alpha=SLOPE
        )
        # phi's inputs go on the sync DMA queue; weights/biases load
        # concurrently on the scalar engine's hardware DGE queue.
        vt_sb = singles.tile([D, N], bf)
        nc.sync.dma_start(out=vt_sb, in_=vt)
        vtq_sb = singles.tile([D, IPC], f32)
        nc.sync.dma_start(out=vtq_sb, in_=vtq)
        w1_sb = singles.tile([D, H1], bf)
        nc.scalar.dma_start(out=w1_sb, in_=w1t2)
        m1_sb = singles.tile([D, 128], bf)
        nc.scalar.dma_start(out=m1_sb, in_=m1t)
        w2_sb = singles.tile([2 * H1, 2 * H2], bf)
        nc.scalar.dma_start(out=w2_sb, in_=w2bd)
        w3_sb = singles.tile([128, 8 * H2], bf)
        nc.scalar.dma_start(out=w3_sb, in_=w3v)
        cb_sb = singles.tile([128, IPC // 2], f32)
        nc.scalar.dma_start(out=cb_sb, in_=cbias)
        b2_sb = singles.tile([128, 1], f32)
        nc.scalar.dma_start(out=b2_sb, in_=b2s)

        phis = {}    # e -> list of 4 phi tiles
        p1s = {}     # e -> [2 x (128,512) psum tiles]
        h1s = {}     # e -> [2 x (128,512) sbuf tiles]
        p2s = {}     # even e -> (128,1024) psum pair tile (e, e+1)
        h2s = {}     # even e -> (128,1024) sbuf pair tile
        lgs = {}     # ib -> (128,512) psum tile
        expos = {}   # ib -> ((128,512) f32 expo tile, (128,1) sums tile)

        def stage_phi(e):
            i0 = 4 * e
            tiles = []
            for k in range(4):
                ph = phip.tile([D, N], bf, tag="phip")
                nc.vector.tensor_scalar(
                    out=ph, in0=vt_sb,
                    scalar1=vtq_sb[:, i0 + k : i0 + k + 1], scalar2=0.0,
                    op0=SUB, op1=MAX,
                )
                tiles.append(ph)
            phis[e] = tiles

        def stage_l1_batch(e):
            # L1 for steps e and e+1 (8 rows) in one same-weight-batched
            # burst: 4x m1 seeds, then 8x w1 accumulates.  Consecutive
            # matmuls alternate PSUM banks (same-bank back-to-back writes
            # serialize) and the PE does only two weight switches.
            tiles = phis.pop(e) + phis.pop(e + 1)
            ps = []
            for t in range(4):
                p1 = p1p.tile([128, N], f32, tag="p1")
                # one M=128 matmul seeds BOTH partition halves with -W1@x
                nc.tensor.matmul(
                    p1, m1_sb, vt_sb,
                    start=True, stop=False, skip_group_check=True,
                )
                ps.append(p1)
            for k in range(2):
                for t in range(4):
                    # per-element has_written semantics make the
                    # seed + per-half accumulate legal.
                    nc.tensor.matmul(
                        ps[t][64 * k : 64 * k + 64, :],
                        w1_sb, tiles[2 * t + k],
                        start=False, stop=True, skip_group_check=True,
                    )
            p1s[e] = ps[0:2]
            p1s[e + 1] = ps[2:4]

        def stage_h1(e):
            ps = p1s.pop(e)
            hs = []
            for half in range(2):
                tp = 2 * e + half  # global pair index
                h1 = h1p.tile([128, N], bf, tag="h1")
                if e % 2 == 1 and half == 1:
                    # offload one of four evacuations to the vector engine
                    nc.vector._custom_dve(
                        PRELU_BIAS, out=h1, in0=ps[half],
                        s0=cb_sb[:, tp : tp + 1], imm2=SLOPE,
                    )
                else:
                    nc.scalar.activation(
                        out=h1, in_=ps[half], func=LR,
                        bias=cb_sb[:, tp : tp + 1], scale=1.0, alpha=SLOPE,
                    )
                hs.append(h1)
            h1s[e] = hs

        def stage_l2_pair(e):
            # L2 for steps e (cols 0:512) and e+1 (cols 512:1024) of one
            # (128,1024) pair tile, interleaved so consecutive matmuls hit
            # different PSUM banks.
            hs0 = h1s.pop(e)
            hs1 = h1s.pop(e + 1)
            p2_new = p2p.tile([128, 2 * N], f32, tag="p2")
            p2s[e] = p2_new
            for half in range(2):
                nc.tensor.matmul(
                    p2_new[64 * half : 64 * half + 64, 0:N],
                    w2_sb, hs0[half], start=True, stop=True,
                )
                nc.tensor.matmul(
                    p2_new[64 * half : 64 * half + 64, N : 2 * N],
                    w2_sb, hs1[half], start=True, stop=True,
                )

        def stage_h2pair(e_even):
            p2 = p2s.pop(e_even)
            h2 = h2p.tile([128, 2 * N], bf, tag="h2")
            nc.scalar.activation(
                out=h2, in_=p2, func=LR, bias=b2_sb, scale=1.0, alpha=SLOPE
            )
            h2s[e_even] = h2

        def stage_l3(e):
            ib, q = divmod(e, QPB)
            if q == 0:
                lg_new = lgp.tile([128, N], f32, tag="lg")
                lgs[ib] = lg_new
            lg = lgs[ib]
            c0, dsh = divmod(q, 8)
            h2 = h2s[e - (e % 2)]
            coff = N * (e % 2)
            nc.tensor.matmul(
                lg[32 * c0 : 32 * c0 + 32, :],
                w3_sb[:, 32 * dsh : 32 * dsh + 32],
                h2[:, coff : coff + N],
                start=(dsh == 0),
                stop=(dsh == 7),
                tile_position=(0, 32 * c0),
            )
            if e % 2 == 1:
                del h2s[e - 1]
            if dsh == 7:
                # this 32-row group's logits are final: exponentiate the slab
                # now so the softmax overlaps the remaining groups' matmuls.
                if c0 == 0:
                    expo_new = postp.tile([128, N], f32, tag="expo")
                    sums_new = postp.tile([128, 1], f32, tag="sums")
                    expos[ib] = (expo_new, sums_new)
                expo, sums = expos[ib]
                nc.scalar.activation(
                    out=expo[32 * c0 : 32 * c0 + 32, :],
                    in_=lg[32 * c0 : 32 * c0 + 32, :],
                    func=EXP,
                    accum_out=sums[32 * c0 : 32 * c0 + 32, :],
                )

        def stage_softmax(ib):
            lgs.pop(ib)
            expo, sums = expos.pop(ib)
            rs = postp.tile([128, 1], f32, tag="rs")
            nc.vector.reciprocal(rs, sums)
            res = postp.tile([128, N], f32, tag="res")
            nc.vector.tensor_scalar_mul(out=res, in0=expo, scalar1=rs)
            nc.sync.dma_start(out=outd[ib * 128 : (ib + 1) * 128, :], in_=res)

        # software-pipelined schedule over step pairs
        for ee in range(-4, NQ + 3, 2):
            if 0 <= ee + 4 < NQ:
                stage_phi(ee + 4)
                stage_phi(ee + 5)
            if 0 <= ee + 2 < NQ:
                stage_l1_batch(ee + 2)
            if 0 <= ee < NQ:
                stage_h1(ee)
                stage_h1(ee + 1)
            if 0 <= ee - 2 < NQ:
                stage_l2_pair(ee - 2)
                stage_h2pair(ee - 2)
            if 0 <= ee - 4 < NQ:
                stage_l3(ee - 4)
                stage_l3(ee - 3)
                for e3 in (ee - 4, ee - 3):
                    if e3 % QPB == QPB - 1:
                        stage_softmax(e3 // QPB)
    return nc


def make_in_maps(v, W1, b1, W2, b2, W3):
    w1t2 = np.ascontiguousarray(2.0 * W1.T).astype(BF16)             # (128, 64)
    m1t = np.ascontiguousarray(
        np.concatenate([-W1.T, -W1.T], axis=1)
    ).astype(BF16)                                                   # (128, 128)
    w1bf = w1t2.astype(np.float32).T / 2.0                           # device W1
    w2bd = np.zeros((2 * H1, 2 * H2), np.float32)
    w2bd[0:H1, 0:H2] = W2.T
    w2bd[H1 : 2 * H1, H2 : 2 * H2] = W2.T
    w2bd = w2bd.astype(BF16)
    w3v = np.zeros((128, 8 * H2), np.float32)
    for dsh in range(8):
        for k in range(4):
            w3v[32 * k : 32 * k + 32, 32 * dsh + 4 * dsh + k] = W3[0, :]
    w3v = w3v.astype(BF16)
    b2st = np.tile(b2, 4).reshape(128, 1).astype(np.float32)

    vts = v.transpose(0, 2, 1)  # (B, D, N)
    in_maps = []
    for c in range(NCORES):
        b, io = c // 2, IPC * (c % 2)
        vt_c = np.ascontiguousarray(vts[b]).astype(BF16)
        vtq_c = np.ascontiguousarray(vt_c[:, io : io + IPC].astype(np.float32))
        # per-row L1 constant: W1@v_i + b1, stacked per pair into one column
        c2 = w1bf @ vtq_c + b1[:, None]                              # (64, IPC)
        cbias_c = np.empty((128, IPC // 2), np.float32)
        cbias_c[0:H1] = c2[:, 0::2]
        cbias_c[H1:128] = c2[:, 1::2]
        in_maps.append(
            {
                "vt": vt_c,
                "vtq": vtq_c,
                "w1t2": w1t2,
                "m1t": m1t,
                "w2bd": w2bd,
                "w3v": w3v,
                "cbias": cbias_c,
                "b2s": b2st,
            }
        )
    return in_maps


_NC_CACHE = [None]


def get_nc():
    if _NC_CACHE[0] is None:
        nc = build_nc()
        nc.finalize()  # runs the Bacc lowering passes (reg alloc, sem split)
        _NC_CACHE[0] = nc
    return _NC_CACHE[0]


def run(inputs, trace=False, **kw):
    nc = get_nc()
    in_maps = make_in_maps(
        inputs["v"], inputs["W1"], inputs["b1"], inputs["W2"], inputs["b2"],
        inputs["W3"],
    )
    res = run_bass_kernel_spmd(nc, in_maps, list(range(NCORES)), trace=trace, **kw)
    out = np.empty((B, N, N), np.float32)
    for c in range(NCORES):
        b, io = c // 2, IPC * (c % 2)
        out[b, io : io + IPC, :] = np.asarray(res.results[c]["out"], np.float32)
    return out, res


def kernel(**inputs):
    out, _ = run(inputs, trace=False)
    return out


# revision 19
# speedup vs baseline: 1.0092x; 1.0092x over previous
"""Trainium2 Bass kernel for the pairwise-MLP adjacency module.

Computes out[b,i,j] = softmax_j( MLP(|v[b,i,:] - v[b,j,:]|) ) where the MLP is
128 -> 64 (leaky 0.1) -> 32 (leaky 0.1) -> 1, implemented as 1x1 convs in the
reference.

Sharding: 8 cores, 2 cores per batch element b (B=4); each core computes 256
of the 512 softmax rows for its b. Weights are replicated, packed host-side.

Per-core dataflow (v3, software-pipelined):
  - DVE: phi_i = relu(VT - v_i) as one fused tensor_scalar (sub + max0).
  - PE L1: z1 = 2*W1@relu(d) - W1@x + (W1@v_i + b1); two rows share a
    (128,512) PSUM tile (partition halves), the -W1@x term is one shared
    M=128 matmul (m1t), and the per-row constant rides the Prelu bias.
  - ACT: leaky-relu + bias fused into every PSUM->SBUF move (Prelu, 0.1).
  - PE L2: block-diag2(W2T) processes 2 rows per matmul; two consecutive
    steps write the two (128,512) halves of a (128,1024) 2-bank PSUM tile.
  - ACT: ONE Prelu evacuates each (128,1024) h2 pair (b2 bias is constant
    across rows, so pairing is legal; amortizes the ScalarE fixed cost).
  - PE L3: 8 shifted zero-padded copies of block-diag4(W3T) accumulate logits
    for 128 rows densely into one PSUM bank (4 col-groups x 8 shifts).
  - ACT: softmax via single Exp with fused row-sum (accum_out); DVE
    reciprocal + scale.  b3 dropped (softmax shift-invariant).
  - The PE stream at step e is [6x L1(e+2), 2x L2(e), L3(e-2)] so every
    matmul's inputs are ready well before issue (minimizes PE stalls, which
    otherwise hold the tensor engine in its low-clock pstate).
"""

import sys

for _p in ("/opt/trn_rl_repo",):
    if _p not in sys.path:
        sys.path.insert(0, _p)

from contextlib import ExitStack

import numpy as np
import ml_dtypes

import concourse.bass as bass
import concourse.bacc as bacc
import concourse.tile as tile
from concourse import mybir
from concourse.bass_utils import run_bass_kernel_spmd

BF16 = ml_dtypes.bfloat16


def _register_prelu_bias_op():
    """Register a custom DVE op computing Prelu(in0 + s0) with slope imm2,
    used to offload part of the PSUM->SBUF h1 evacuation from the (busy)
    scalar engine to the vector engine.  Uses the documented custom-DVE
    extension point (concourse.dve_ops.OPS); the uops_sha pin is computed
    from the same lower() call DveOp.compile uses."""
    import numpy as np
    from concourse import dve_ops as dvo
    from concourse.dve_spec import Spec, Src0, C0, C2, maxx
    from concourse.dve_spec import lower as dve_lower
    from concourse.dve_uop import DveOpSpec

    name = "PRELU_BIAS_ANT"
    for op in dvo.OPS:
        if op.name == name:
            return op

    def _ref(in0, in1, c0, c1, c2):
        u = np.asarray(in0, np.float32) + c0
        return np.maximum(u, u * c2)

    u = Src0 + C0
    spec = Spec(body=maxx(u, u * C2), reference=_ref)
    op = dvo.DveOp(name, spec, subdim=False, uops_sha={})
    dvo.OPS.append(op)
    dvo.CUSTOM_DVE_SPECS[name] = spec
    dvo._SUB_OPCODE_FOR_NAME[name] = dvo._CUSTOM_DVE_ROW_BASE + len(dvo.OPS) - 1
    for ver in ("v3", "v4"):
        r = DveOpSpec(
            name=name,
            opcode=dvo.get_dve_sub_opcode(name),
            uops=dve_lower(spec, ver=ver),
            rd1_en=dvo.has_src1(spec),
        )
        op.uops_sha[ver] = r.sha(ver)
    return op


PRELU_BIAS = _register_prelu_bias_op()

B, N, D = 4, 512, 128
H1, H2 = 64, 32
SLOPE = 0.1
NCORES = 8
IPC = B * N // NCORES      # 256 rows per core
NQ = IPC // 4              # 64 steps of 4 rows
QPB = NQ // 2              # 32 steps per softmax batch of 128 rows


def build_nc():
    f32 = mybir.dt.float32
    bf = mybir.dt.bfloat16
    nc = bacc.Bacc("TRN2", target_bir_lowering=False, debug=False)

    vt = nc.dram_tensor("vt", [D, N], bf, kind="ExternalInput").ap()
    vtq = nc.dram_tensor("vtq", [D, IPC], f32, kind="ExternalInput").ap()
    w1t2 = nc.dram_tensor("w1t2", [D, H1], bf, kind="ExternalInput").ap()
    m1t = nc.dram_tensor("m1t", [D, 128], bf, kind="ExternalInput").ap()
    w2bd = nc.dram_tensor("w2bd", [2 * H1, 2 * H2], bf, kind="ExternalInput").ap()
    w3v = nc.dram_tensor("w3v", [128, 8 * H2], bf, kind="ExternalInput").ap()
    cbias = nc.dram_tensor("cbias", [128, IPC // 2], f32, kind="ExternalInput").ap()
    b2s = nc.dram_tensor("b2s", [128, 1], f32, kind="ExternalInput").ap()
    outd = nc.dram_tensor("out", [IPC, N], f32, kind="ExternalOutput").ap()

    LR = mybir.ActivationFunctionType.Prelu  # parametric relu: reads alpha
    EXP = mybir.ActivationFunctionType.Exp
    SUB = mybir.AluOpType.subtract
    MAX = mybir.AluOpType.max

    with tile.TileContext(nc) as tc, ExitStack() as ctx:
        singles = ctx.enter_context(tc.tile_pool(name="singles", bufs=1))
        phip = ctx.enter_context(tc.tile_pool(name="phip", bufs=16))
        h1p = ctx.enter_context(tc.tile_pool(name="h1p", bufs=6))
        h2p = ctx.enter_context(tc.tile_pool(name="h2p", bufs=2))
        p1p = ctx.enter_context(tc.tile_pool(name="p1p", bufs=4, space="PSUM"))
        p2p = ctx.enter_context(tc.tile_pool(name="p2p", bufs=1, space="PSUM"))
        lgp = ctx.enter_context(tc.tile_pool(name="lgp", bufs=2, space="PSUM"))
        postp = ctx.enter_context(tc.tile_pool(name="postp", bufs=2))

        # dummy 1-col Prelu: pulls the ACT table load into the prologue where
        # it overlaps the input DMAs instead of stalling the first h1 act.
        warm0 = singles.tile([128, 1], f32)
        nc.vector.memset(warm0, 0.0)
        warm = singles.tile([128, 1], f32)
        nc.scalar.activation(
            out=warm, in_=warm0, func=LR, scale=1.0, alpha=SLOPE
        )
        # phi's inputs go on the sync DMA queue; weights/biases load
        # concurrently on the scalar engine's hardware DGE queue.
        vt_sb = singles.tile([D, N], bf)
        nc.sync.dma_start(out=vt_sb, in_=vt)
        vtq_sb = singles.tile([D, IPC], f32)
        nc.sync.dma_start(out=vtq_sb, in_=vtq)
        w1_sb = singles.tile([D, H1], bf)
        nc.scalar.dma_start(out=w1_sb, in_=w1t2)
        m1_sb = singles.tile([D, 128], bf)
        nc.scalar.dma_start(out=m1_sb, in_=m1t)
        w2_sb = singles.tile([2 * H1, 2 * H2], bf)
        nc.scalar.dma_start(out=w2_sb, in_=w2bd)
        w3_sb = singles.tile([128, 8 * H2], bf)
        nc.scalar.dma_start(out=w3_sb, in_=w3v)
        cb_sb = singles.tile([128, IPC // 2], f32)
        nc.scalar.dma_start(out=cb_sb, in_=cbias)
        b2_sb = singles.tile([128, 1], f32)
        nc.scalar.dma_start(out=b2_sb, in_=b2s)

        phis = {}    # e -> list of 4 phi tiles
        p1s = {}     # e -> [2 x (128,512) psum tiles]
        h1s = {}     # e -> [2 x (128,512) sbuf tiles]
        p2s = {}     # even e -> (128,1024) psum pair tile (e, e+1)
        h2s = {}     # even e -> (128,1024) sbuf pair tile
        lgs = {}     # ib -> (128,512) psum tile
        expos = {}   # ib -> ((128,512) f32 expo tile, (128,1) sums tile)

        def stage_phi(e):
            i0 = 4 * e
            tiles = []
            for k in range(4):
                ph = phip.tile([D, N], bf, tag="phip")
                nc.vector.tensor_scalar(
                    out=ph, in0=vt_sb,
                    scalar1=vtq_sb[:, i0 + k : i0 + k + 1], scalar2=0.0,
                    op0=SUB, op1=MAX,
                )
                tiles.append(ph)
            phis[e] = tiles

        def stage_l1_batch(e):
            # L1 for steps e and e+1 (8 rows) in one same-weight-batched
            # burst: 4x m1 seeds, then 8x w1 accumulates.  Consecutive
            # matmuls alternate PSUM banks (same-bank back-to-back writes
            # serialize) and the PE does only two weight switches.
            tiles = phis.pop(e) + phis.pop(e + 1)
            ps = []
            for t in range(4):
                p1 = p1p.tile([128, N], f32, tag="p1")
                # one M=128 matmul seeds BOTH partition halves with -W1@x
                nc.tensor.matmul(
                    p1, m1_sb, vt_sb,
                    start=True, stop=False, skip_group_check=True,
                )
                ps.append(p1)
            for k in range(2):
                for t in range(4):
                    # per-element has_written semantics make the
                    # seed + per-half accumulate legal.
                    nc.tensor.matmul(
                        ps[t][64 * k : 64 * k + 64, :],
                        w1_sb, tiles[2 * t + k],
                        start=False, stop=True, skip_group_check=True,
                    )
            p1s[e] = ps[0:2]
            p1s[e + 1] = ps[2:4]

        def stage_h1(e):
            ps = p1s.pop(e)
            hs = []
            for half in range(2):
                tp = 2 * e + half  # global pair index
                h1 = h1p.tile([128, N], bf, tag="h1")
                if e % 2 == 1 and half == 1:
                    # offload one of four evacuations to the vector engine
                    nc.vector._custom_dve(
                        PRELU_BIAS, out=h1, in0=ps[half],
                        s0=cb_sb[:, tp : tp + 1], imm2=SLOPE,
                    )
                else:
                    nc.scalar.activation(
                        out=h1, in_=ps[half], func=LR,
                        bias=cb_sb[:, tp : tp + 1], scale=1.0, alpha=SLOPE,
                    )
                hs.append(h1)
            h1s[e] = hs

        def stage_l2_pair(e):
            # L2 for steps e (cols 0:512) and e+1 (cols 512:1024) of one
            # (128,1024) pair tile, interleaved so consecutive matmuls hit
            # different PSUM banks.
            hs0 = h1s.pop(e)
            hs1 = h1s.pop(e + 1)
            p2_new = p2p.tile([128, 2 * N], f32, tag="p2")
            p2s[e] = p2_new
            for half in range(2):
                nc.tensor.matmul(
                    p2_new[64 * half : 64 * half + 64, 0:N],
                    w2_sb, hs0[half], start=True, stop=True,
                )
                nc.tensor.matmul(
                    p2_new[64 * half : 64 * half + 64, N : 2 * N],
                    w2_sb, hs1[half], start=True, stop=True,
                )

        def stage_h2pair(e_even):
            p2 = p2s.pop(e_even)
            h2 = h2p.tile([128, 2 * N], bf, tag="h2")
            nc.scalar.activation(
                out=h2, in_=p2, func=LR, bias=b2_sb, scale=1.0, alpha=SLOPE
            )
            h2s[e_even] = h2

        def stage_l3(e):
            ib, q = divmod(e, QPB)
            if q == 0:
                lg_new = lgp.tile([128, N], f32, tag="lg")
                lgs[ib] = lg_new
            lg = lgs[ib]
            c0, dsh = divmod(q, 8)
            h2 = h2s[e - (e % 2)]
            coff = N * (e % 2)
            nc.tensor.matmul(
                lg[32 * c0 : 32 * c0 + 32, :],
                w3_sb[:, 32 * dsh : 32 * dsh + 32],
                h2[:, coff : coff + N],
                start=(dsh == 0),
                stop=(dsh == 7),
                tile_position=(0, 32 * c0),
            )
            if e % 2 == 1:
                del h2s[e - 1]
            if dsh == 7:
                # this 32-row group's logits are final: run its whole softmax
                # slab (exp+sum, reciprocal, scale, store) now so it overlaps
                # the remaining groups' matmuls and shrinks the kernel tail.
                if c0 == 0:
                    expo_new = postp.tile([128, N], f32, tag="expo")
                    sums_new = postp.tile([128, 1], f32, tag="sums")
                    rs_new = postp.tile([128, 1], f32, tag="rs")
                    res_new = postp.tile([128, N], f32, tag="res")
                    expos[ib] = (expo_new, sums_new, rs_new, res_new)
                expo, sums, rs, res = expos[ib]
                sl = slice(32 * c0, 32 * c0 + 32)
                nc.scalar.activation(
                    out=expo[sl, :], in_=lg[sl, :], func=EXP,
                    accum_out=sums[sl, :],
                )
                nc.vector.reciprocal(rs[sl, :], sums[sl, :])
                nc.vector.tensor_scalar_mul(
                    out=res[sl, :], in0=expo[sl, :], scalar1=rs[sl, :]
                )
                nc.sync.dma_start(
                    out=outd[ib * 128 + 32 * c0 : ib * 128 + 32 * c0 + 32, :],
                    in_=res[sl, :],
                )

        def stage_softmax(ib):
            lgs.pop(ib)
            expos.pop(ib)

        # software-pipelined schedule over step pairs
        for ee in range(-4, NQ + 3, 2):
            if 0 <= ee + 4 < NQ:
                stage_phi(ee + 4)
                stage_phi(ee + 5)
            if 0 <= ee + 2 < NQ:
                stage_l1_batch(ee + 2)
            if 0 <= ee < NQ:
                stage_h1(ee)
                stage_h1(ee + 1)
            if 0 <= ee - 2 < NQ:
                stage_l2_pair(ee - 2)
                stage_h2pair(ee - 2)
            if 0 <= ee - 4 < NQ:
                stage_l3(ee - 4)
                stage_l3(ee - 3)
                for e3 in (ee - 4, ee - 3):
                    if e3 % QPB == QPB - 1:
                        stage_softmax(e3 // QPB)
    return nc


def make_in_maps(v, W1, b1, W2, b2, W3):
    w1t2 = np.ascontiguousarray(2.0 * W1.T).astype(BF16)             # (128, 64)
    m1t = np.ascontiguousarray(
        np.concatenate([-W1.T, -W1.T], axis=1)
    ).astype(BF16)                                                   # (128, 128)
    w1bf = w1t2.astype(np.float32).T / 2.0                           # device W1
    w2bd = np.zeros((2 * H1, 2 * H2), np.float32)
    w2bd[0:H1, 0:H2] = W2.T
    w2bd[H1 : 2 * H1, H2 : 2 * H2] = W2.T
    w2bd = w2bd.astype(BF16)
    w3v = np.zeros((128, 8 * H2), np.float32)
    for dsh in range(8):
        for k in range(4):
            w3v[32 * k : 32 * k + 32, 32 * dsh + 4 * dsh + k] = W3[0, :]
    w3v = w3v.astype(BF16)
    b2st = np.tile(b2, 4).reshape(128, 1).astype(np.float32)

    vts = v.transpose(0, 2, 1)  # (B, D, N)
    in_maps = []
    for c in range(NCORES):
        b, io = c // 2, IPC * (c % 2)
        vt_c = np.ascontiguousarray(vts[b]).astype(BF16)
        vtq_c = np.ascontiguousarray(vt_c[:, io : io + IPC].astype(np.float32))
        # per-row L1 constant: W1@v_i + b1, stacked per pair into one column
        c2 = w1bf @ vtq_c + b1[:, None]                              # (64, IPC)
        cbias_c = np.empty((128, IPC // 2), np.float32)
        cbias_c[0:H1] = c2[:, 0::2]
        cbias_c[H1:128] = c2[:, 1::2]
        in_maps.append(
            {
                "vt": vt_c,
                "vtq": vtq_c,
                "w1t2": w1t2,
                "m1t": m1t,
                "w2bd": w2bd,
                "w3v": w3v,
                "cbias": cbias_c,
                "b2s": b2st,
            }
        )
    return in_maps


_NC_CACHE = [None]


def get_nc():
    if _NC_CACHE[0] is None:
        nc = build_nc()
        nc.finalize()  # runs the Bacc lowering passes (reg alloc, sem split)
        _NC_CACHE[0] = nc
    return _NC_CACHE[0]


def run(inputs, trace=False, **kw):
    nc = get_nc()
    in_maps = make_in_maps(
        inputs["v"], inputs["W1"], inputs["b1"], inputs["W2"], inputs["b2"],
        inputs["W3"],
    )
    res = run_bass_kernel_spmd(nc, in_maps, list(range(NCORES)), trace=trace, **kw)
    out = np.empty((B, N, N), np.float32)
    for c in range(NCORES):
        b, io = c // 2, IPC * (c % 2)
        out[b, io : io + IPC, :] = np.asarray(res.results[c]["out"], np.float32)
    return out, res


def kernel(**inputs):
    out, _ = run(inputs, trace=False)
    return out


# revision 26
# speedup vs baseline: 1.1093x; 1.0992x over previous
"""Trainium2 Bass kernel for the pairwise-MLP adjacency module.

Computes out[b,i,j] = softmax_j( MLP(|v[b,i,:] - v[b,j,:]|) ) where the MLP is
128 -> 64 (leaky 0.1) -> 32 (leaky 0.1) -> 1, implemented as 1x1 convs in the
reference.

Sharding: 8 cores, 2 cores per batch element b (B=4); each core computes 256
of the 512 softmax rows for its b. Weights are replicated, packed host-side.

Per-core dataflow (v3, software-pipelined):
  - DVE: phi_i = relu(VT - v_i) as one fused tensor_scalar (sub + max0).
  - PE L1: z1 = 2*W1@relu(d) - W1@x + (W1@v_i + b1); two rows share a
    (128,512) PSUM tile (partition halves), the -W1@x term is one shared
    M=128 matmul (m1t), and the per-row constant rides the Prelu bias.
  - ACT: leaky-relu + bias fused into every PSUM->SBUF move (Prelu, 0.1).
  - PE L2: block-diag2(W2T) processes 2 rows per matmul; two consecutive
    steps write the two (128,512) halves of a (128,1024) 2-bank PSUM tile.
  - ACT: ONE Prelu evacuates each (128,1024) h2 pair (b2 bias is constant
    across rows, so pairing is legal; amortizes the ScalarE fixed cost).
  - PE L3: 8 shifted zero-padded copies of block-diag4(W3T) accumulate logits
    for 128 rows densely into one PSUM bank (4 col-groups x 8 shifts).
  - ACT: softmax via single Exp with fused row-sum (accum_out); DVE
    reciprocal + scale.  b3 dropped (softmax shift-invariant).
  - The PE stream at step e is [6x L1(e+2), 2x L2(e), L3(e-2)] so every
    matmul's inputs are ready well before issue (minimizes PE stalls, which
    otherwise hold the tensor engine in its low-clock pstate).
"""

import sys

for _p in ("/opt/trn_rl_repo",):
    if _p not in sys.path:
        sys.path.insert(0, _p)

from contextlib import ExitStack

import numpy as np
import ml_dtypes

import concourse.bass as bass
import concourse.bacc as bacc
import concourse.tile as tile
from concourse import mybir
from concourse.bass_utils import run_bass_kernel_spmd

BF16 = ml_dtypes.bfloat16


def _register_prelu_bias_op():
    """Register a custom DVE op computing Prelu(in0 + s0) with slope imm2,
    used to offload part of the PSUM->SBUF h1 evacuation from the (busy)
    scalar engine to the vector engine.  Uses the documented custom-DVE
    extension point (concourse.dve_ops.OPS); the uops_sha pin is computed
    from the same lower() call DveOp.compile uses."""
    import numpy as np
    from concourse import dve_ops as dvo
    from concourse.dve_spec import Spec, Src0, C0, C2, maxx
    from concourse.dve_spec import lower as dve_lower
    from concourse.dve_uop import DveOpSpec

    name = "PRELU_BIAS_ANT"
    for op in dvo.OPS:
        if op.name == name:
            return op

    def _ref(in0, in1, c0, c1, c2):
        u = np.asarray(in0, np.float32) + c0
        return np.maximum(u, u * c2)

    u = Src0 + C0
    spec = Spec(body=maxx(u, u * C2), reference=_ref)
    op = dvo.DveOp(name, spec, subdim=False, uops_sha={})
    dvo.OPS.append(op)
    dvo.CUSTOM_DVE_SPECS[name] = spec
    dvo._SUB_OPCODE_FOR_NAME[name] = dvo._CUSTOM_DVE_ROW_BASE + len(dvo.OPS) - 1
    for ver in ("v3", "v4"):
        r = DveOpSpec(
            name=name,
            opcode=dvo.get_dve_sub_opcode(name),
            uops=dve_lower(spec, ver=ver),
            rd1_en=dvo.has_src1(spec),
        )
        op.uops_sha[ver] = r.sha(ver)
    return op


PRELU_BIAS = _register_prelu_bias_op()

B, N, D = 4, 512, 128
H1, H2 = 64, 32
SLOPE = 0.1
NCORES = 8
IPC = B * N // NCORES      # 256 rows per core
NQ = IPC // 4              # 64 steps of 4 rows
QPB = NQ // 2              # 32 steps per softmax batch of 128 rows


def build_nc():
    f32 = mybir.dt.float32
    bf = mybir.dt.bfloat16
    nc = bacc.Bacc("TRN2", target_bir_lowering=False, debug=False)

    vt = nc.dram_tensor("vt", [D, N], bf, kind="ExternalInput").ap()
    vtq = nc.dram_tensor("vtq", [D, IPC], f32, kind="ExternalInput").ap()
    w1t2 = nc.dram_tensor("w1t2", [D, H1], bf, kind="ExternalInput").ap()
    m1t = nc.dram_tensor("m1t", [D, 128], bf, kind="ExternalInput").ap()
    w2bd = nc.dram_tensor("w2bd", [2 * H1, 2 * H2], bf, kind="ExternalInput").ap()
    w3v = nc.dram_tensor("w3v", [128, 8 * H2], bf, kind="ExternalInput").ap()
    cbias = nc.dram_tensor("cbias", [128, IPC // 2], f32, kind="ExternalInput").ap()
    b2s = nc.dram_tensor("b2s", [128, 1], f32, kind="ExternalInput").ap()
    idm = nc.dram_tensor("idm", [128, 128], f32, kind="ExternalInput").ap()
    outd = nc.dram_tensor("out", [IPC, N], f32, kind="ExternalOutput").ap()

    LR = mybir.ActivationFunctionType.Prelu  # parametric relu: reads alpha
    EXP = mybir.ActivationFunctionType.Exp
    SUB = mybir.AluOpType.subtract
    MAX = mybir.AluOpType.max

    with tile.TileContext(nc) as tc, ExitStack() as ctx:
        singles = ctx.enter_context(tc.tile_pool(name="singles", bufs=1))
        phip = ctx.enter_context(tc.tile_pool(name="phip", bufs=16))
        h1p = ctx.enter_context(tc.tile_pool(name="h1p", bufs=6))
        h2p = ctx.enter_context(tc.tile_pool(name="h2p", bufs=2))
        p1p = ctx.enter_context(tc.tile_pool(name="p1p", bufs=4, space="PSUM"))
        p2p = ctx.enter_context(tc.tile_pool(name="p2p", bufs=1, space="PSUM"))
        lgp = ctx.enter_context(tc.tile_pool(name="lgp", bufs=2, space="PSUM"))
        postp = ctx.enter_context(tc.tile_pool(name="postp", bufs=2))

        # dummy 1-col Prelu: pulls the ACT table load into the prologue where
        # it overlaps the input DMAs instead of stalling the first h1 act.
        warm0 = singles.tile([128, 1], f32)
        nc.vector.memset(warm0, 0.0)
        warm = singles.tile([128, 1], f32)
        nc.scalar.activation(
            out=warm, in_=warm0, func=LR, scale=1.0, alpha=SLOPE
        )
        # phi's inputs go on the sync DMA queue; weights/biases load
        # concurrently on the scalar engine's hardware DGE queue.
        vt_sb = singles.tile([D, N], bf)
        nc.sync.dma_start(out=vt_sb, in_=vt)
        vtq_sb = singles.tile([D, IPC], f32)
        nc.sync.dma_start(out=vtq_sb, in_=vtq)
        w1_sb = singles.tile([D, H1], bf)
        nc.scalar.dma_start(out=w1_sb, in_=w1t2)
        m1_sb = singles.tile([D, 128], bf)
        nc.scalar.dma_start(out=m1_sb, in_=m1t)
        w2_sb = singles.tile([2 * H1, 2 * H2], bf)
        nc.scalar.dma_start(out=w2_sb, in_=w2bd)
        w3_sb = singles.tile([128, 8 * H2], bf)
        nc.scalar.dma_start(out=w3_sb, in_=w3v)
        cb_sb = singles.tile([128, IPC // 2], f32)
        nc.scalar.dma_start(out=cb_sb, in_=cbias)
        b2_sb = singles.tile([128, 1], f32)
        nc.scalar.dma_start(out=b2_sb, in_=b2s)
        id_sb = singles.tile([128, 128], f32)
        nc.scalar.dma_start(out=id_sb, in_=idm)
        tr_sb = singles.tile([128, 128], f32)

        phis = {}    # e -> list of 4 phi tiles
        p1s = {}     # e -> [2 x (128,512) psum tiles]
        h1s = {}     # e -> [2 x (128,512) sbuf tiles]
        p2s = {}     # even e -> (128,1024) psum pair tile (e, e+1)
        h2s = {}     # even e -> (128,1024) sbuf pair tile
        lgs = {}     # ib -> (128,512) psum tile
        expos = {}   # ib -> ((128,512) f32 expo tile, (128,1) sums tile)

        def _phase(e):
            # batch A (steps < QPB) covers all 512 j-columns; batch B covers
            # only local columns 128..512 — its missing 128x128 logit block
            # is the transpose of batch A's columns 128..256 (the host
            # permutes each core's columns so this holds on every core).
            return (0, N) if e < QPB else (128, N - 128)

        def stage_phi(e):
            noff, nn = _phase(e)
            i0 = 4 * e
            tiles = []
            for k in range(4):
                ph = phip.tile([D, N], bf, tag="phip")
                nc.vector.tensor_scalar(
                    out=ph[:, 0:nn], in0=vt_sb[:, noff : noff + nn],
                    scalar1=vtq_sb[:, i0 + k : i0 + k + 1], scalar2=0.0,
                    op0=SUB, op1=MAX,
                )
                tiles.append(ph)
            phis[e] = tiles

        def stage_l1_batch(e):
            # L1 for steps e and e+1 (8 rows) in one same-weight-batched
            # burst: 4x m1 seeds, then 8x w1 accumulates.  Consecutive
            # matmuls alternate PSUM banks (same-bank back-to-back writes
            # serialize) and the PE does only two weight switches.
            noff, nn = _phase(e)
            tiles = phis.pop(e) + phis.pop(e + 1)
            ps = []
            for t in range(4):
                p1 = p1p.tile([128, N], f32, tag="p1")
                # one M=128 matmul seeds BOTH partition halves with -W1@x
                nc.tensor.matmul(
                    p1[:, 0:nn], m1_sb, vt_sb[:, noff : noff + nn],
                    start=True, stop=False, skip_group_check=True,
                )
                ps.append(p1)
            for k in range(2):
                for t in range(4):
                    # per-element has_written semantics make the
                    # seed + per-half accumulate legal.
                    nc.tensor.matmul(
                        ps[t][64 * k : 64 * k + 64, 0:nn],
                        w1_sb, tiles[2 * t + k][:, 0:nn],
                        start=False, stop=True, skip_group_check=True,
                    )
            p1s[e] = ps[0:2]
            p1s[e + 1] = ps[2:4]

        def stage_h1(e):
            noff, nn = _phase(e)
            ps = p1s.pop(e)
            hs = []
            for half in range(2):
                tp = 2 * e + half  # global pair index
                h1 = h1p.tile([128, N], bf, tag="h1")
                if e % 2 == 1 and half == 1:
                    # offload one of four evacuations to the vector engine
                    nc.vector._custom_dve(
                        PRELU_BIAS, out=h1[:, 0:nn], in0=ps[half][:, 0:nn],
                        s0=cb_sb[:, tp : tp + 1], imm2=SLOPE,
                    )
                else:
                    nc.scalar.activation(
                        out=h1[:, 0:nn], in_=ps[half][:, 0:nn], func=LR,
                        bias=cb_sb[:, tp : tp + 1], scale=1.0, alpha=SLOPE,
                    )
                hs.append(h1)
            h1s[e] = hs

        def stage_l2_pair(e):
            # L2 for steps e (bank 0) and e+1 (bank 1) of one (128,2,512)
            # pair tile, interleaved so consecutive matmuls hit different
            # PSUM banks.
            noff, nn = _phase(e)
            hs0 = h1s.pop(e)
            hs1 = h1s.pop(e + 1)
            p2_new = p2p.tile([128, 2, N], f32, tag="p2")
            p2s[e] = p2_new
            for half in range(2):
                nc.tensor.matmul(
                    p2_new[64 * half : 64 * half + 64, 0, 0:nn],
                    w2_sb, hs0[half][:, 0:nn], start=True, stop=True,
                )
                nc.tensor.matmul(
                    p2_new[64 * half : 64 * half + 64, 1, 0:nn],
                    w2_sb, hs1[half][:, 0:nn], start=True, stop=True,
                )

        def stage_h2pair(e_even):
            noff, nn = _phase(e_even)
            p2 = p2s.pop(e_even)
            h2 = h2p.tile([128, 2, N], bf, tag="h2")
            nc.scalar.activation(
                out=h2[:, :, 0:nn], in_=p2[:, :, 0:nn], func=LR,
                bias=b2_sb, scale=1.0, alpha=SLOPE,
            )
            h2s[e_even] = h2

        def stage_l3(e):
            noff, nn = _phase(e)
            ib, q = divmod(e, QPB)
            if q == 0:
                lg_new = lgp.tile([128, N], f32, tag="lg")
                lgs[ib] = lg_new
            lg = lgs[ib]
            if e == QPB:
                # fill batch B's skipped block with the transposed A block
                # BEFORE any L3 write touches this bank (a later start=True
                # in the same bank would reset earlier writes).
                nc.tensor.matmul(
                    lg[:, 0:128], tr_sb, id_sb,
                    is_transpose=True, skip_group_check=True,
                )
            c0, dsh = divmod(q, 8)
            h2 = h2s[e - (e % 2)]
            nc.tensor.matmul(
                lg[32 * c0 : 32 * c0 + 32, noff : noff + nn],
                w3_sb[:, 32 * dsh : 32 * dsh + 32],
                h2[:, e % 2, 0:nn],
                start=(dsh == 0),
                stop=(dsh == 7),
                tile_position=(0, 32 * c0),
            )
            if e % 2 == 1:
                del h2s[e - 1]
            if e == QPB - 1:
                # batch A logits complete: stage the a[A-rows, B-rows] block
                # in SBUF for the symmetry transpose.
                nc.vector.tensor_copy(tr_sb, lgs[0][:, 128:256])
            if dsh == 7:
                # this 32-row group's logits are final: run its whole softmax
                # slab (exp+sum, reciprocal, scale, store) now so it overlaps
                # the remaining groups' matmuls and shrinks the kernel tail.
                if c0 == 0:
                    expo_new = postp.tile([128, N], f32, tag="expo")
                    sums_new = postp.tile([128, 1], f32, tag="sums")
                    rs_new = postp.tile([128, 1], f32, tag="rs")
                    res_new = postp.tile([128, N], f32, tag="res")
                    expos[ib] = (expo_new, sums_new, rs_new, res_new)
                expo, sums, rs, res = expos[ib]
                sl = slice(32 * c0, 32 * c0 + 32)
                nc.scalar.activation(
                    out=expo[sl, :], in_=lg[sl, :], func=EXP,
                    accum_out=sums[sl, :],
                )
                nc.vector.reciprocal(rs[sl, :], sums[sl, :])
                nc.vector.tensor_scalar_mul(
                    out=res[sl, :], in0=expo[sl, :], scalar1=rs[sl, :]
                )
                nc.sync.dma_start(
                    out=outd[ib * 128 + 32 * c0 : ib * 128 + 32 * c0 + 32, :],
                    in_=res[sl, :],
                )

        def stage_softmax(ib):
            lgs.pop(ib)
            expos.pop(ib)

        # software-pipelined schedule over step pairs
        for ee in range(-4, NQ + 3, 2):
            if 0 <= ee + 4 < NQ:
                stage_phi(ee + 4)
                stage_phi(ee + 5)
            if 0 <= ee + 2 < NQ:
                stage_l1_batch(ee + 2)
            if 0 <= ee < NQ:
                stage_h1(ee)
                stage_h1(ee + 1)
            if 0 <= ee - 2 < NQ:
                stage_l2_pair(ee - 2)
                stage_h2pair(ee - 2)
            if 0 <= ee - 4 < NQ:
                stage_l3(ee - 4)
                stage_l3(ee - 3)
                for e3 in (ee - 4, ee - 3):
                    if e3 % QPB == QPB - 1:
                        stage_softmax(e3 // QPB)
    return nc


def make_in_maps(v, W1, b1, W2, b2, W3):
    w1t2 = np.ascontiguousarray(2.0 * W1.T).astype(BF16)             # (128, 64)
    m1t = np.ascontiguousarray(
        np.concatenate([-W1.T, -W1.T], axis=1)
    ).astype(BF16)                                                   # (128, 128)
    w1bf = w1t2.astype(np.float32).T / 2.0                           # device W1
    w2bd = np.zeros((2 * H1, 2 * H2), np.float32)
    w2bd[0:H1, 0:H2] = W2.T
    w2bd[H1 : 2 * H1, H2 : 2 * H2] = W2.T
    w2bd = w2bd.astype(BF16)
    w3v = np.zeros((128, 8 * H2), np.float32)
    for dsh in range(8):
        for k in range(4):
            w3v[32 * k : 32 * k + 32, 32 * dsh + 4 * dsh + k] = W3[0, :]
    w3v = w3v.astype(BF16)
    b2st = np.tile(b2, 4).reshape(128, 1).astype(np.float32)

    idm = np.eye(128, dtype=np.float32)
    vts = v.transpose(0, 2, 1)  # (B, D, N)
    in_maps = []
    for c in range(NCORES):
        b, rowmap, perm = core_geometry(c)
        vt_c = np.ascontiguousarray(vts[b][:, perm]).astype(BF16)
        vtq_c = np.ascontiguousarray(vt_c[:, 0:IPC].astype(np.float32))
        # per-row L1 constant: W1@v_i + b1, stacked per pair into one column
        c2 = w1bf @ vtq_c + b1[:, None]                              # (64, IPC)
        cbias_c = np.empty((128, IPC // 2), np.float32)
        cbias_c[0:H1] = c2[:, 0::2]
        cbias_c[H1:128] = c2[:, 1::2]
        in_maps.append(
            {
                "vt": vt_c,
                "vtq": vtq_c,
                "w1t2": w1t2,
                "m1t": m1t,
                "w2bd": w2bd,
                "w3v": w3v,
                "cbias": cbias_c,
                "b2s": b2st,
                "idm": idm,
            }
        )
    return in_maps


def core_geometry(c):
    """Per-core row ownership and column permutation.

    Each core's first 128 rows (batch A) compute all 512 columns; its second
    128 rows (batch B) skip local columns 0:128 — which are exactly batch A's
    rows, so the missing logit block is recovered on-device by transposing
    a[A-rows, B-rows] (the pairwise MLP is symmetric in (i, j))."""
    b, half = c // 2, c % 2
    if half == 0:
        perm = np.arange(N)
        rowmap = np.arange(0, IPC)
    else:
        perm = np.concatenate(
            [np.arange(384, 512), np.arange(256, 384), np.arange(0, 256)]
        )
        rowmap = perm[0:IPC]
    return b, rowmap, perm


_NC_CACHE = [None]


def get_nc():
    if _NC_CACHE[0] is None:
        nc = build_nc()
        nc.finalize()  # runs the Bacc lowering passes (reg alloc, sem split)
        _NC_CACHE[0] = nc
    return _NC_CACHE[0]


def run(inputs, trace=False, **kw):
    nc = get_nc()
    in_maps = make_in_maps(
        inputs["v"], inputs["W1"], inputs["b1"], inputs["W2"], inputs["b2"],
        inputs["W3"],
    )
    res = run_bass_kernel_spmd(nc, in_maps, list(range(NCORES)), trace=trace, **kw)
    out = np.empty((B, N, N), np.float32)
    for c in range(NCORES):
        b, rowmap, perm = core_geometry(c)
        out[b][np.ix_(rowmap, perm)] = np.asarray(
            res.results[c]["out"], np.float32
        )
    return out, res


def kernel(**inputs):
    out, _ = run(inputs, trace=False)
    return out


# revision 32
# speedup vs baseline: 1.1833x; 1.0667x over previous
"""Trainium2 Bass kernel for the pairwise-MLP adjacency module.

Computes out[b,i,j] = softmax_j( MLP(|v[b,i,:] - v[b,j,:]|) ) where the MLP is
128 -> 64 (leaky 0.1) -> 32 (leaky 0.1) -> 1, implemented as 1x1 convs in the
reference.

Sharding: 8 cores, 2 cores per batch element b (B=4); each core computes 256
of the 512 softmax rows for its b. Weights are replicated, packed host-side.

Per-core dataflow (v3, software-pipelined):
  - DVE: phi_i = relu(VT - v_i) as one fused tensor_scalar (sub + max0).
  - PE L1: z1 = 2*W1@relu(d) - W1@x + (W1@v_i + b1); two rows share a
    (128,512) PSUM tile (partition halves), the -W1@x term is one shared
    M=128 matmul (m1t), and the per-row constant rides the Prelu bias.
  - ACT: leaky-relu + bias fused into every PSUM->SBUF move (Prelu, 0.1).
  - PE L2: block-diag2(W2T) processes 2 rows per matmul; two consecutive
    steps write the two (128,512) halves of a (128,1024) 2-bank PSUM tile.
  - ACT: ONE Prelu evacuates each (128,1024) h2 pair (b2 bias is constant
    across rows, so pairing is legal; amortizes the ScalarE fixed cost).
  - PE L3: 8 shifted zero-padded copies of block-diag4(W3T) accumulate logits
    for 128 rows densely into one PSUM bank (4 col-groups x 8 shifts).
  - ACT: softmax via single Exp with fused row-sum (accum_out); DVE
    reciprocal + scale.  b3 dropped (softmax shift-invariant).
  - The PE stream at step e is [6x L1(e+2), 2x L2(e), L3(e-2)] so every
    matmul's inputs are ready well before issue (minimizes PE stalls, which
    otherwise hold the tensor engine in its low-clock pstate).
"""

import sys

for _p in ("/opt/trn_rl_repo",):
    if _p not in sys.path:
        sys.path.insert(0, _p)

from contextlib import ExitStack

import numpy as np
import ml_dtypes

import concourse.bass as bass
import concourse.bacc as bacc
import concourse.tile as tile
from concourse import mybir
from concourse.bass_utils import run_bass_kernel_spmd

BF16 = ml_dtypes.bfloat16


def _register_dve_op(name, spec):
    """Register a custom DVE op via the documented extension point
    (concourse.dve_ops.OPS); the uops_sha pin is computed from the same
    lower() call DveOp.compile uses."""
    from concourse import dve_ops as dvo
    from concourse.dve_spec import lower as dve_lower
    from concourse.dve_uop import DveOpSpec

    for op in dvo.OPS:
        if op.name == name:
            return op
    op = dvo.DveOp(name, spec, subdim=False, uops_sha={})
    dvo.OPS.append(op)
    dvo.CUSTOM_DVE_SPECS[name] = spec
    dvo._SUB_OPCODE_FOR_NAME[name] = dvo._CUSTOM_DVE_ROW_BASE + len(dvo.OPS) - 1
    for ver in ("v3", "v4"):
        r = DveOpSpec(
            name=name,
            opcode=dvo.get_dve_sub_opcode(name),
            uops=dve_lower(spec, ver=ver),
            rd1_en=dvo.has_src1(spec),
        )
        op.uops_sha[ver] = r.sha(ver)
    return op


def _make_prelu_ops():
    """Prelu(in0 + in1 + s0) with slope imm2 — offloads part of the
    PSUM->SBUF h1 evacuation from the (busy) scalar engine to the vector
    engine, folding in the -W1@x correction (in1) so those PSUM tiles skip
    their m1 seed matmul entirely."""
    import numpy as np
    from concourse.dve_spec import Spec, Src0, Src1, C0, C2, maxx

    def _ref_r(in0, in1, c0, c1, c2):
        u = np.asarray(in0, np.float32) + np.asarray(in1, np.float32) + c0
        return np.maximum(u, u * c2)

    ur = Src0 + Src1 + C0
    spec_r = Spec(body=maxx(ur, ur * C2), reference=_ref_r)
    return _register_dve_op("PRELU_BIAS_R_ANT", spec_r)


PRELU_BIAS_R = _make_prelu_ops()

B, N, D = 4, 512, 128
H1, H2 = 64, 32
SLOPE = 0.1
NCORES = 8
IPC = B * N // NCORES      # 256 rows per core
NQ = IPC // 4              # 64 steps of 4 rows
QPB = NQ // 2              # 32 steps per softmax batch of 128 rows


def build_nc():
    f32 = mybir.dt.float32
    bf = mybir.dt.bfloat16
    nc = bacc.Bacc("TRN2", target_bir_lowering=False, debug=False)

    vt = nc.dram_tensor("vt", [D, N], bf, kind="ExternalInput").ap()
    vtq = nc.dram_tensor("vtq", [D, IPC], f32, kind="ExternalInput").ap()
    w1t2 = nc.dram_tensor("w1t2", [D, H1], bf, kind="ExternalInput").ap()
    m1t = nc.dram_tensor("m1t", [D, 128], bf, kind="ExternalInput").ap()
    w2bd = nc.dram_tensor("w2bd", [2 * H1, 2 * H2], bf, kind="ExternalInput").ap()
    w3v = nc.dram_tensor("w3v", [128, 8 * H2], bf, kind="ExternalInput").ap()
    cbias = nc.dram_tensor("cbias", [128, IPC // 2], f32, kind="ExternalInput").ap()
    b2s = nc.dram_tensor("b2s", [128, 1], f32, kind="ExternalInput").ap()
    idm = nc.dram_tensor("idm", [128, 128], f32, kind="ExternalInput").ap()
    outd = nc.dram_tensor("out", [IPC, N], f32, kind="ExternalOutput").ap()

    LR = mybir.ActivationFunctionType.Prelu  # parametric relu: reads alpha
    EXP = mybir.ActivationFunctionType.Exp
    SUB = mybir.AluOpType.subtract
    MAX = mybir.AluOpType.max

    with tile.TileContext(nc) as tc, ExitStack() as ctx:
        singles = ctx.enter_context(tc.tile_pool(name="singles", bufs=1))
        phip = ctx.enter_context(tc.tile_pool(name="phip", bufs=16))
        h1p = ctx.enter_context(tc.tile_pool(name="h1p", bufs=6))
        h2p = ctx.enter_context(tc.tile_pool(name="h2p", bufs=2))
        p1p = ctx.enter_context(tc.tile_pool(name="p1p", bufs=4, space="PSUM"))
        p2p = ctx.enter_context(tc.tile_pool(name="p2p", bufs=1, space="PSUM"))
        lgp = ctx.enter_context(tc.tile_pool(name="lgp", bufs=2, space="PSUM"))
        postp = ctx.enter_context(tc.tile_pool(name="postp", bufs=2))

        # dummy 1-col Prelu: pulls the ACT table load into the prologue where
        # it overlaps the input DMAs instead of stalling the first h1 act.
        warm0 = singles.tile([128, 1], f32)
        nc.vector.memset(warm0, 0.0)
        warm = singles.tile([128, 1], f32)
        nc.scalar.activation(
            out=warm, in_=warm0, func=LR, scale=1.0, alpha=SLOPE
        )
        # phi's inputs go on the sync DMA queue; weights/biases load
        # concurrently on the scalar engine's hardware DGE queue.
        vt_sb = singles.tile([D, N], bf)
        nc.sync.dma_start(out=vt_sb, in_=vt)
        vtq_sb = singles.tile([D, IPC], f32)
        nc.sync.dma_start(out=vtq_sb, in_=vtq)
        w1_sb = singles.tile([D, H1], bf)
        nc.scalar.dma_start(out=w1_sb, in_=w1t2)
        m1_sb = singles.tile([D, 128], bf)
        nc.scalar.dma_start(out=m1_sb, in_=m1t)
        w2_sb = singles.tile([2 * H1, 2 * H2], bf)
        nc.scalar.dma_start(out=w2_sb, in_=w2bd)
        w3_sb = singles.tile([128, 8 * H2], bf)
        nc.scalar.dma_start(out=w3_sb, in_=w3v)
        cb_sb = singles.tile([128, IPC // 2], f32)
        nc.scalar.dma_start(out=cb_sb, in_=cbias)
        b2_sb = singles.tile([128, 1], f32)
        nc.scalar.dma_start(out=b2_sb, in_=b2s)
        id_sb = singles.tile([128, 128], f32)
        nc.scalar.dma_start(out=id_sb, in_=idm)
        tr_sb = singles.tile([128, 128], f32)
        # -W1@x correction, computed once and fed to the custom DVE
        # evacuation op as its second stream (frees one seed matmul per batch)
        rcc_ps = p1p.tile([128, N], f32, tag="p1", name="rcc_ps")
        nc.tensor.matmul(rcc_ps, m1_sb, vt_sb, start=True, stop=True)
        rcc_sb = singles.tile([128, N], f32)
        nc.vector.tensor_copy(rcc_sb, rcc_ps)

        phis = {}    # e -> list of 4 phi tiles
        p1s = {}     # e -> [2 x (128,512) psum tiles]
        h1s = {}     # e -> [2 x (128,512) sbuf tiles]
        p2s = {}     # even e -> (128,1024) psum pair tile (e, e+1)
        h2s = {}     # even e -> (128,1024) sbuf pair tile
        lgs = {}     # ib -> (128,512) psum tile
        expos = {}   # ib -> ((128,512) f32 expo tile, (128,1) sums tile)

        def _phase(e):
            # batch A (steps < QPB) covers all 512 j-columns; batch B covers
            # only local columns 128..512 — its missing 128x128 logit block
            # is the transpose of batch A's columns 128..256 (the host
            # permutes each core's columns so this holds on every core).
            return (0, N) if e < QPB else (128, N - 128)

        def stage_phi(e):
            noff, nn = _phase(e)
            i0 = 4 * e
            tiles = []
            for k in range(4):
                ph = phip.tile([D, N], bf, tag="phip")
                nc.vector.tensor_scalar(
                    out=ph[:, 0:nn], in0=vt_sb[:, noff : noff + nn],
                    scalar1=vtq_sb[:, i0 + k : i0 + k + 1], scalar2=0.0,
                    op0=SUB, op1=MAX,
                )
                tiles.append(ph)
            phis[e] = tiles

        def stage_l1_batch(e):
            # L1 for steps e and e+1 (8 rows) in one same-weight-batched
            # burst: 4x m1 seeds, then 8x w1 accumulates.  Consecutive
            # matmuls alternate PSUM banks (same-bank back-to-back writes
            # serialize) and the PE does only two weight switches.
            noff, nn = _phase(e)
            tiles = phis.pop(e) + phis.pop(e + 1)
            ps = []
            for t in range(4):
                p1 = p1p.tile([128, N], f32, tag="p1")
                if t < 3:
                    # one M=128 matmul seeds BOTH partition halves with
                    # -W1@x; tile 3 skips the seed — its evacuation runs on
                    # the vector engine with the correction as in1.
                    nc.tensor.matmul(
                        p1[:, 0:nn], m1_sb, vt_sb[:, noff : noff + nn],
                        start=True, stop=False, skip_group_check=True,
                    )
                ps.append(p1)
            for k in range(2):
                for t in range(4):
                    # per-element has_written semantics make the
                    # seed + per-half accumulate legal.
                    nc.tensor.matmul(
                        ps[t][64 * k : 64 * k + 64, 0:nn],
                        w1_sb, tiles[2 * t + k][:, 0:nn],
                        start=(t == 3), stop=True, skip_group_check=True,
                    )
            p1s[e] = ps[0:2]
            p1s[e + 1] = ps[2:4]

        def stage_h1(e):
            noff, nn = _phase(e)
            ps = p1s.pop(e)
            hs = []
            for half in range(2):
                tp = 2 * e + half  # global pair index
                h1 = h1p.tile([128, N], bf, tag="h1")
                if e % 2 == 1 and half == 1:
                    # offload one of four evacuations to the vector engine,
                    # folding in the -W1@x correction this tile skipped
                    nc.vector._custom_dve(
                        PRELU_BIAS_R, out=h1[:, 0:nn], in0=ps[half][:, 0:nn],
                        in1=rcc_sb[:, noff : noff + nn],
                        s0=cb_sb[:, tp : tp + 1], imm2=SLOPE,
                    )
                else:
                    nc.scalar.activation(
                        out=h1[:, 0:nn], in_=ps[half][:, 0:nn], func=LR,
                        bias=cb_sb[:, tp : tp + 1], scale=1.0, alpha=SLOPE,
                    )
                hs.append(h1)
            h1s[e] = hs

        def stage_l2_pair(e):
            # L2 for steps e (bank 0) and e+1 (bank 1) of one (128,2,512)
            # pair tile, interleaved so consecutive matmuls hit different
            # PSUM banks.
            noff, nn = _phase(e)
            hs0 = h1s.pop(e)
            hs1 = h1s.pop(e + 1)
            p2_new = p2p.tile([128, 2, N], f32, tag="p2")
            p2s[e] = p2_new
            for half in range(2):
                nc.tensor.matmul(
                    p2_new[64 * half : 64 * half + 64, 0, 0:nn],
                    w2_sb, hs0[half][:, 0:nn], start=True, stop=True,
                )
                nc.tensor.matmul(
                    p2_new[64 * half : 64 * half + 64, 1, 0:nn],
                    w2_sb, hs1[half][:, 0:nn], start=True, stop=True,
                )

        def stage_h2pair(e_even):
            noff, nn = _phase(e_even)
            p2 = p2s.pop(e_even)
            h2 = h2p.tile([128, 2, N], bf, tag="h2")
            nc.scalar.activation(
                out=h2[:, :, 0:nn], in_=p2[:, :, 0:nn], func=LR,
                bias=b2_sb, scale=1.0, alpha=SLOPE,
            )
            h2s[e_even] = h2

        def stage_l3(e):
            noff, nn = _phase(e)
            ib, q = divmod(e, QPB)
            if q == 0:
                lg_new = lgp.tile([128, N], f32, tag="lg")
                lgs[ib] = lg_new
            lg = lgs[ib]
            if e == QPB:
                # fill batch B's skipped block with the transposed A block
                # BEFORE any L3 write touches this bank (a later start=True
                # in the same bank would reset earlier writes).
                nc.tensor.matmul(
                    lg[:, 0:128], tr_sb, id_sb,
                    is_transpose=True, skip_group_check=True,
                )
            c0, dsh = divmod(q, 8)
            h2 = h2s[e - (e % 2)]
            nc.tensor.matmul(
                lg[32 * c0 : 32 * c0 + 32, noff : noff + nn],
                w3_sb[:, 32 * dsh : 32 * dsh + 32],
                h2[:, e % 2, 0:nn],
                start=(dsh == 0),
                stop=(dsh == 7),
                tile_position=(0, 32 * c0),
            )
            if e % 2 == 1:
                del h2s[e - 1]
            if e == QPB - 1:
                # batch A logits complete: stage the a[A-rows, B-rows] block
                # in SBUF for the symmetry transpose.
                nc.vector.tensor_copy(tr_sb, lgs[0][:, 128:256])
            if dsh == 7 and c0 % 2 == 1:
                # two 32-row groups' logits are final: run their softmax
                # slab (exp+sum, reciprocal, scale, store) now so it overlaps
                # the remaining groups' matmuls and shrinks the kernel tail.
                # (a 64-row slab costs the same as 32 — cost is column-driven)
                if c0 == 1:
                    expo_new = postp.tile([128, N], f32, tag="expo")
                    sums_new = postp.tile([128, 1], f32, tag="sums")
                    rs_new = postp.tile([128, 1], f32, tag="rs")
                    res_new = postp.tile([128, N], f32, tag="res")
                    expos[ib] = (expo_new, sums_new, rs_new, res_new)
                expo, sums, rs, res = expos[ib]
                sl = slice(32 * (c0 - 1), 32 * c0 + 32)
                nc.scalar.activation(
                    out=expo[sl, :], in_=lg[sl, :], func=EXP,
                    accum_out=sums[sl, :],
                )
                nc.vector.reciprocal(rs[sl, :], sums[sl, :])
                nc.vector.tensor_scalar_mul(
                    out=res[sl, :], in0=expo[sl, :], scalar1=rs[sl, :]
                )
                nc.sync.dma_start(
                    out=outd[
                        ib * 128 + 32 * (c0 - 1) : ib * 128 + 32 * c0 + 32, :
                    ],
                    in_=res[sl, :],
                )

        def stage_softmax(ib):
            lgs.pop(ib)
            expos.pop(ib)

        # software-pipelined schedule over step pairs
        for ee in range(-4, NQ + 3, 2):
            if 0 <= ee + 4 < NQ:
                stage_phi(ee + 4)
                stage_phi(ee + 5)
            if 0 <= ee + 2 < NQ:
                stage_l1_batch(ee + 2)
            if 0 <= ee < NQ:
                stage_h1(ee)
                stage_h1(ee + 1)
            if 0 <= ee - 2 < NQ:
                stage_l2_pair(ee - 2)
                stage_h2pair(ee - 2)
            if 0 <= ee - 4 < NQ:
                stage_l3(ee - 4)
                stage_l3(ee - 3)
                for e3 in (ee - 4, ee - 3):
                    if e3 % QPB == QPB - 1:
                        stage_softmax(e3 // QPB)
    return nc


def make_in_maps(v, W1, b1, W2, b2, W3):
    w1t2 = np.ascontiguousarray(2.0 * W1.T).astype(BF16)             # (128, 64)
    m1t = np.ascontiguousarray(
        np.concatenate([-W1.T, -W1.T], axis=1)
    ).astype(BF16)                                                   # (128, 128)
    w1bf = w1t2.astype(np.float32).T / 2.0                           # device W1
    w2bd = np.zeros((2 * H1, 2 * H2), np.float32)
    w2bd[0:H1, 0:H2] = W2.T
    w2bd[H1 : 2 * H1, H2 : 2 * H2] = W2.T
    w2bd = w2bd.astype(BF16)
    w3v = np.zeros((128, 8 * H2), np.float32)
    for dsh in range(8):
        for k in range(4):
            w3v[32 * k : 32 * k + 32, 32 * dsh + 4 * dsh + k] = W3[0, :]
    w3v = w3v.astype(BF16)
    b2st = np.tile(b2, 4).reshape(128, 1).astype(np.float32)

    idm = np.eye(128, dtype=np.float32)
    vts = v.transpose(0, 2, 1)  # (B, D, N)
    in_maps = []
    for c in range(NCORES):
        b, rowmap, perm = core_geometry(c)
        vt_c = np.ascontiguousarray(vts[b][:, perm]).astype(BF16)
        vtq_c = np.ascontiguousarray(vt_c[:, 0:IPC].astype(np.float32))
        # per-row L1 constant: W1@v_i + b1, stacked per pair into one column
        c2 = w1bf @ vtq_c + b1[:, None]                              # (64, IPC)
        cbias_c = np.empty((128, IPC // 2), np.float32)
        cbias_c[0:H1] = c2[:, 0::2]
        cbias_c[H1:128] = c2[:, 1::2]
        in_maps.append(
            {
                "vt": vt_c,
                "vtq": vtq_c,
                "w1t2": w1t2,
                "m1t": m1t,
                "w2bd": w2bd,
                "w3v": w3v,
                "cbias": cbias_c,
                "b2s": b2st,
                "idm": idm,
            }
        )
    return in_maps


def core_geometry(c):
    """Per-core row ownership and column permutation.

    Each core's first 128 rows (batch A) compute all 512 columns; its second
    128 rows (batch B) skip local columns 0:128 — which are exactly batch A's
    rows, so the missing logit block is recovered on-device by transposing
    a[A-rows, B-rows] (the pairwise MLP is symmetric in (i, j))."""
    b, half = c // 2, c % 2
    if half == 0:
        perm = np.arange(N)
        rowmap = np.arange(0, IPC)
    else:
        perm = np.concatenate(
            [np.arange(384, 512), np.arange(256, 384), np.arange(0, 256)]
        )
        rowmap = perm[0:IPC]
    return b, rowmap, perm


_NC_CACHE = [None]


def get_nc():
    if _NC_CACHE[0] is None:
        nc = build_nc()
        nc.finalize()  # runs the Bacc lowering passes (reg alloc, sem split)
        _NC_CACHE[0] = nc
    return _NC_CACHE[0]


def run(inputs, trace=False, **kw):
    nc = get_nc()
    in_maps = make_in_maps(
        inputs["v"], inputs["W1"], inputs["b1"], inputs["W2"], inputs["b2"],
        inputs["W3"],
    )
    res = run_bass_kernel_spmd(nc, in_maps, list(range(NCORES)), trace=trace, **kw)
    out = np.empty((B, N, N), np.float32)
    for c in range(NCORES):
        b, rowmap, perm = core_geometry(c)
        out[b][np.ix_(rowmap, perm)] = np.asarray(
            res.results[c]["out"], np.float32
        )
    return out, res


def kernel(**inputs):
    out, _ = run(inputs, trace=False)
    return out


# revision 36
# speedup vs baseline: 1.1934x; 1.0085x over previous
"""Trainium2 Bass kernel for the pairwise-MLP adjacency module.

Computes out[b,i,j] = softmax_j( MLP(|v[b,i,:] - v[b,j,:]|) ) where the MLP is
128 -> 64 (leaky 0.1) -> 32 (leaky 0.1) -> 1, implemented as 1x1 convs in the
reference.

Sharding: 8 cores, 2 cores per batch element b (B=4); each core computes 256
of the 512 softmax rows for its b. Weights are replicated, packed host-side.

Per-core dataflow (v3, software-pipelined):
  - DVE: phi_i = relu(VT - v_i) as one fused tensor_scalar (sub + max0).
  - PE L1: z1 = 2*W1@relu(d) - W1@x + (W1@v_i + b1); two rows share a
    (128,512) PSUM tile (partition halves), the -W1@x term is one shared
    M=128 matmul (m1t), and the per-row constant rides the Prelu bias.
  - ACT: leaky-relu + bias fused into every PSUM->SBUF move (Prelu, 0.1).
  - PE L2: block-diag2(W2T) processes 2 rows per matmul; two consecutive
    steps write the two (128,512) halves of a (128,1024) 2-bank PSUM tile.
  - ACT: ONE Prelu evacuates each (128,1024) h2 pair (b2 bias is constant
    across rows, so pairing is legal; amortizes the ScalarE fixed cost).
  - PE L3: 8 shifted zero-padded copies of block-diag4(W3T) accumulate logits
    for 128 rows densely into one PSUM bank (4 col-groups x 8 shifts).
  - ACT: softmax via single Exp with fused row-sum (accum_out); DVE
    reciprocal + scale.  b3 dropped (softmax shift-invariant).
  - The PE stream at step e is [6x L1(e+2), 2x L2(e), L3(e-2)] so every
    matmul's inputs are ready well before issue (minimizes PE stalls, which
    otherwise hold the tensor engine in its low-clock pstate).
"""

import sys

for _p in ("/opt/trn_rl_repo",):
    if _p not in sys.path:
        sys.path.insert(0, _p)

from contextlib import ExitStack

import numpy as np
import ml_dtypes

import concourse.bass as bass
import concourse.bacc as bacc
import concourse.tile as tile
from concourse import mybir
from concourse.bass_utils import run_bass_kernel_spmd

BF16 = ml_dtypes.bfloat16


def _register_dve_op(name, spec):
    """Register a custom DVE op via the documented extension point
    (concourse.dve_ops.OPS); the uops_sha pin is computed from the same
    lower() call DveOp.compile uses."""
    from concourse import dve_ops as dvo
    from concourse.dve_spec import lower as dve_lower
    from concourse.dve_uop import DveOpSpec

    for op in dvo.OPS:
        if op.name == name:
            return op
    op = dvo.DveOp(name, spec, subdim=False, uops_sha={})
    dvo.OPS.append(op)
    dvo.CUSTOM_DVE_SPECS[name] = spec
    dvo._SUB_OPCODE_FOR_NAME[name] = dvo._CUSTOM_DVE_ROW_BASE + len(dvo.OPS) - 1
    for ver in ("v3", "v4"):
        r = DveOpSpec(
            name=name,
            opcode=dvo.get_dve_sub_opcode(name),
            uops=dve_lower(spec, ver=ver),
            rd1_en=dvo.has_src1(spec),
        )
        op.uops_sha[ver] = r.sha(ver)
    return op


def _make_prelu_ops():
    """Prelu(in0 + in1 + s0) with slope imm2 — offloads part of the
    PSUM->SBUF h1 evacuation from the (busy) scalar engine to the vector
    engine, folding in the -W1@x correction (in1) so those PSUM tiles skip
    their m1 seed matmul entirely."""
    import numpy as np
    from concourse.dve_spec import Spec, Src0, Src1, C0, C2, maxx

    def _ref_r(in0, in1, c0, c1, c2):
        u = np.asarray(in0, np.float32) + np.asarray(in1, np.float32) + c0
        return np.maximum(u, u * c2)

    ur = Src0 + Src1 + C0
    spec_r = Spec(body=maxx(ur, ur * C2), reference=_ref_r)
    return _register_dve_op("PRELU_BIAS_R_ANT", spec_r)


PRELU_BIAS_R = _make_prelu_ops()

B, N, D = 4, 512, 128
H1, H2 = 64, 32
SLOPE = 0.1
NCORES = 8
IPC = B * N // NCORES      # 256 rows per core
NQ = IPC // 4              # 64 steps of 4 rows
QPB = NQ // 2              # 32 steps per softmax batch of 128 rows


def build_nc():
    f32 = mybir.dt.float32
    bf = mybir.dt.bfloat16
    nc = bacc.Bacc("TRN2", target_bir_lowering=False, debug=False)

    vt = nc.dram_tensor("vt", [D, N], bf, kind="ExternalInput").ap()
    vtq = nc.dram_tensor("vtq", [D, IPC], f32, kind="ExternalInput").ap()
    w1t2 = nc.dram_tensor("w1t2", [D, H1], bf, kind="ExternalInput").ap()
    m1t = nc.dram_tensor("m1t", [D, 128], bf, kind="ExternalInput").ap()
    w2bd = nc.dram_tensor("w2bd", [2 * H1, 2 * H2], bf, kind="ExternalInput").ap()
    w3v = nc.dram_tensor("w3v", [128, 8 * H2], bf, kind="ExternalInput").ap()
    cbias = nc.dram_tensor("cbias", [128, IPC // 2], f32, kind="ExternalInput").ap()
    b2s = nc.dram_tensor("b2s", [128, 1], f32, kind="ExternalInput").ap()
    idm = nc.dram_tensor("idm", [128, 128], f32, kind="ExternalInput").ap()
    outd = nc.dram_tensor("out", [IPC, N], f32, kind="ExternalOutput").ap()

    LR = mybir.ActivationFunctionType.Prelu  # parametric relu: reads alpha
    EXP = mybir.ActivationFunctionType.Exp
    SUB = mybir.AluOpType.subtract
    MAX = mybir.AluOpType.max

    with tile.TileContext(nc) as tc, ExitStack() as ctx:
        singles = ctx.enter_context(tc.tile_pool(name="singles", bufs=1))
        phip = ctx.enter_context(tc.tile_pool(name="phip", bufs=16))
        h1p = ctx.enter_context(tc.tile_pool(name="h1p", bufs=6))
        h2p = ctx.enter_context(tc.tile_pool(name="h2p", bufs=2))
        p1p = ctx.enter_context(tc.tile_pool(name="p1p", bufs=4, space="PSUM"))
        p2p = ctx.enter_context(tc.tile_pool(name="p2p", bufs=1, space="PSUM"))
        lgp = ctx.enter_context(tc.tile_pool(name="lgp", bufs=2, space="PSUM"))
        postp = ctx.enter_context(tc.tile_pool(name="postp", bufs=2))

        # dummy 1-col Prelu: pulls the ACT table load into the prologue where
        # it overlaps the input DMAs instead of stalling the first h1 act.
        warm0 = singles.tile([128, 1], f32)
        nc.vector.memset(warm0, 0.0)
        warm = singles.tile([128, 1], f32)
        nc.scalar.activation(
            out=warm, in_=warm0, func=LR, scale=1.0, alpha=SLOPE
        )
        # phi's inputs go on the sync DMA queue; weights/biases load
        # concurrently on the scalar engine's hardware DGE queue.
        vt_sb = singles.tile([D, N], bf)
        nc.sync.dma_start(out=vt_sb, in_=vt)
        vtq_sb = singles.tile([D, IPC], f32)
        nc.sync.dma_start(out=vtq_sb, in_=vtq)
        # queue order = first-consumer order: m1/w1 feed the first matmuls,
        # cbias/b2s the first activations; w2/w3/idm are needed much later.
        m1_sb = singles.tile([D, 128], bf)
        nc.scalar.dma_start(out=m1_sb, in_=m1t)
        w1_sb = singles.tile([D, H1], bf)
        nc.scalar.dma_start(out=w1_sb, in_=w1t2)
        cb_sb = singles.tile([128, IPC // 2], f32)
        nc.scalar.dma_start(out=cb_sb, in_=cbias)
        b2_sb = singles.tile([128, 1], f32)
        nc.scalar.dma_start(out=b2_sb, in_=b2s)
        w2_sb = singles.tile([2 * H1, 2 * H2], bf)
        nc.scalar.dma_start(out=w2_sb, in_=w2bd)
        w3_sb = singles.tile([128, 8 * H2], bf)
        nc.scalar.dma_start(out=w3_sb, in_=w3v)
        id_sb = singles.tile([128, 128], f32)
        nc.scalar.dma_start(out=id_sb, in_=idm)
        tr_sb = singles.tile([128, 128], f32)
        # -W1@x correction, computed once and fed to the custom DVE
        # evacuation op as its second stream (frees one seed matmul per batch)
        rcc_ps = p1p.tile([128, N], f32, tag="p1", name="rcc_ps")
        nc.tensor.matmul(rcc_ps, m1_sb, vt_sb, start=True, stop=True)
        rcc_sb = singles.tile([128, N], f32)
        nc.vector.tensor_copy(rcc_sb, rcc_ps)

        phis = {}    # e -> list of 4 phi tiles
        p1s = {}     # e -> [2 x (128,512) psum tiles]
        h1s = {}     # e -> [2 x (128,512) sbuf tiles]
        p2s = {}     # even e -> (128,1024) psum pair tile (e, e+1)
        h2s = {}     # even e -> (128,1024) sbuf pair tile
        lgs = {}     # ib -> (128,512) psum tile
        expos = {}   # ib -> ((128,512) f32 expo tile, (128,1) sums tile)

        def _phase(e):
            # batch A (steps < QPB) covers all 512 j-columns; batch B covers
            # only local columns 128..512 — its missing 128x128 logit block
            # is the transpose of batch A's columns 128..256 (the host
            # permutes each core's columns so this holds on every core).
            return (0, N) if e < QPB else (128, N - 128)

        def _h1_on_dve(e, half):
            # steady state: one of four evacuations on the vector engine;
            # pipeline drain (no more phi work -> idle DVE): both odd halves,
            # relieving the scalar engine on the critical tail path.
            return half == 1 and (e % 2 == 1 or e >= NQ - 6)

        def stage_phi(e):
            noff, nn = _phase(e)
            i0 = 4 * e
            tiles = []
            for k in range(4):
                ph = phip.tile([D, N], bf, tag="phip")
                nc.vector.tensor_scalar(
                    out=ph[:, 0:nn], in0=vt_sb[:, noff : noff + nn],
                    scalar1=vtq_sb[:, i0 + k : i0 + k + 1], scalar2=0.0,
                    op0=SUB, op1=MAX,
                )
                tiles.append(ph)
            phis[e] = tiles

        def stage_l1_batch(e):
            # L1 for steps e and e+1 (8 rows) in one same-weight-batched
            # burst: 4x m1 seeds, then 8x w1 accumulates.  Consecutive
            # matmuls alternate PSUM banks (same-bank back-to-back writes
            # serialize) and the PE does only two weight switches.
            noff, nn = _phase(e)
            tiles = phis.pop(e) + phis.pop(e + 1)
            ps = []
            for t in range(4):
                p1 = p1p.tile([128, N], f32, tag="p1")
                if not _h1_on_dve(e + t // 2, t % 2):
                    # one M=128 matmul seeds BOTH partition halves with
                    # -W1@x; DVE-evacuated tiles skip the seed — their
                    # evacuation op applies the correction as in1.
                    nc.tensor.matmul(
                        p1[:, 0:nn], m1_sb, vt_sb[:, noff : noff + nn],
                        start=True, stop=False, skip_group_check=True,
                    )
                ps.append(p1)
            for k in range(2):
                for t in range(4):
                    # per-element has_written semantics make the
                    # seed + per-half accumulate legal.
                    nc.tensor.matmul(
                        ps[t][64 * k : 64 * k + 64, 0:nn],
                        w1_sb, tiles[2 * t + k][:, 0:nn],
                        start=_h1_on_dve(e + t // 2, t % 2), stop=True,
                        skip_group_check=True,
                    )
            p1s[e] = ps[0:2]
            p1s[e + 1] = ps[2:4]

        def stage_h1(e):
            noff, nn = _phase(e)
            ps = p1s.pop(e)
            hs = []
            for half in range(2):
                tp = 2 * e + half  # global pair index
                h1 = h1p.tile([128, N], bf, tag="h1")
                if _h1_on_dve(e, half):
                    # offload this evacuation to the vector engine,
                    # folding in the -W1@x correction this tile skipped
                    nc.vector._custom_dve(
                        PRELU_BIAS_R, out=h1[:, 0:nn], in0=ps[half][:, 0:nn],
                        in1=rcc_sb[:, noff : noff + nn],
                        s0=cb_sb[:, tp : tp + 1], imm2=SLOPE,
                    )
                else:
                    nc.scalar.activation(
                        out=h1[:, 0:nn], in_=ps[half][:, 0:nn], func=LR,
                        bias=cb_sb[:, tp : tp + 1], scale=1.0, alpha=SLOPE,
                    )
                hs.append(h1)
            h1s[e] = hs

        def stage_l2_pair(e):
            # L2 for steps e (bank 0) and e+1 (bank 1) of one (128,2,512)
            # pair tile, interleaved so consecutive matmuls hit different
            # PSUM banks.
            noff, nn = _phase(e)
            hs0 = h1s.pop(e)
            hs1 = h1s.pop(e + 1)
            p2_new = p2p.tile([128, 2, N], f32, tag="p2")
            p2s[e] = p2_new
            for half in range(2):
                nc.tensor.matmul(
                    p2_new[64 * half : 64 * half + 64, 0, 0:nn],
                    w2_sb, hs0[half][:, 0:nn], start=True, stop=True,
                )
                nc.tensor.matmul(
                    p2_new[64 * half : 64 * half + 64, 1, 0:nn],
                    w2_sb, hs1[half][:, 0:nn], start=True, stop=True,
                )

        def stage_h2pair(e_even):
            noff, nn = _phase(e_even)
            p2 = p2s.pop(e_even)
            h2 = h2p.tile([128, 2, N], bf, tag="h2")
            nc.scalar.activation(
                out=h2[:, :, 0:nn], in_=p2[:, :, 0:nn], func=LR,
                bias=b2_sb, scale=1.0, alpha=SLOPE,
            )
            h2s[e_even] = h2

        def stage_l3(e):
            noff, nn = _phase(e)
            ib, q = divmod(e, QPB)
            if q == 0:
                lg_new = lgp.tile([128, N], f32, tag="lg")
                lgs[ib] = lg_new
            lg = lgs[ib]
            if e == QPB:
                # fill batch B's skipped block with the transposed A block
                # BEFORE any L3 write touches this bank (a later start=True
                # in the same bank would reset earlier writes).
                nc.tensor.matmul(
                    lg[:, 0:128], tr_sb, id_sb,
                    is_transpose=True, skip_group_check=True,
                )
            c0, dsh = divmod(q, 8)
            h2 = h2s[e - (e % 2)]
            nc.tensor.matmul(
                lg[32 * c0 : 32 * c0 + 32, noff : noff + nn],
                w3_sb[:, 32 * dsh : 32 * dsh + 32],
                h2[:, e % 2, 0:nn],
                start=(dsh == 0),
                stop=(dsh == 7),
                tile_position=(0, 32 * c0),
            )
            if e % 2 == 1:
                del h2s[e - 1]
            if e == QPB - 1:
                # batch A logits complete: stage the a[A-rows, B-rows] block
                # in SBUF for the symmetry transpose.
                nc.vector.tensor_copy(tr_sb, lgs[0][:, 128:256])
            if dsh == 7 and c0 % 2 == 1:
                # two 32-row groups' logits are final: run their softmax
                # slab (exp+sum, reciprocal, scale, store) now so it overlaps
                # the remaining groups' matmuls and shrinks the kernel tail.
                # (a 64-row slab costs the same as 32 — cost is column-driven)
                if c0 == 1:
                    expo_new = postp.tile([128, N], f32, tag="expo")
                    sums_new = postp.tile([128, 1], f32, tag="sums")
                    rs_new = postp.tile([128, 1], f32, tag="rs")
                    res_new = postp.tile([128, N], f32, tag="res")
                    expos[ib] = (expo_new, sums_new, rs_new, res_new)
                expo, sums, rs, res = expos[ib]
                sl = slice(32 * (c0 - 1), 32 * c0 + 32)
                nc.scalar.activation(
                    out=expo[sl, :], in_=lg[sl, :], func=EXP,
                    accum_out=sums[sl, :],
                )
                nc.vector.reciprocal(rs[sl, :], sums[sl, :])
                nc.vector.tensor_scalar_mul(
                    out=res[sl, :], in0=expo[sl, :], scalar1=rs[sl, :]
                )
                nc.sync.dma_start(
                    out=outd[
                        ib * 128 + 32 * (c0 - 1) : ib * 128 + 32 * c0 + 32, :
                    ],
                    in_=res[sl, :],
                )

        def stage_softmax(ib):
            lgs.pop(ib)
            expos.pop(ib)

        # software-pipelined schedule over step pairs
        for ee in range(-4, NQ + 3, 2):
            if 0 <= ee + 4 < NQ:
                stage_phi(ee + 4)
                stage_phi(ee + 5)
            if 0 <= ee + 2 < NQ:
                stage_l1_batch(ee + 2)
            if 0 <= ee < NQ:
                stage_h1(ee)
                stage_h1(ee + 1)
            if 0 <= ee - 2 < NQ:
                stage_l2_pair(ee - 2)
                stage_h2pair(ee - 2)
            if 0 <= ee - 4 < NQ:
                stage_l3(ee - 4)
                stage_l3(ee - 3)
                for e3 in (ee - 4, ee - 3):
                    if e3 % QPB == QPB - 1:
                        stage_softmax(e3 // QPB)
    return nc


def make_in_maps(v, W1, b1, W2, b2, W3):
    w1t2 = np.ascontiguousarray(2.0 * W1.T).astype(BF16)             # (128, 64)
    m1t = np.ascontiguousarray(
        np.concatenate([-W1.T, -W1.T], axis=1)
    ).astype(BF16)                                                   # (128, 128)
    w1bf = w1t2.astype(np.float32).T / 2.0                           # device W1
    w2bd = np.zeros((2 * H1, 2 * H2), np.float32)
    w2bd[0:H1, 0:H2] = W2.T
    w2bd[H1 : 2 * H1, H2 : 2 * H2] = W2.T
    w2bd = w2bd.astype(BF16)
    w3v = np.zeros((128, 8 * H2), np.float32)
    for dsh in range(8):
        for k in range(4):
            w3v[32 * k : 32 * k + 32, 32 * dsh + 4 * dsh + k] = W3[0, :]
    w3v = w3v.astype(BF16)
    b2st = np.tile(b2, 4).reshape(128, 1).astype(np.float32)

    idm = np.eye(128, dtype=np.float32)
    vts = v.transpose(0, 2, 1)  # (B, D, N)
    in_maps = []
    for c in range(NCORES):
        b, rowmap, perm = core_geometry(c)
        vt_c = np.ascontiguousarray(vts[b][:, perm]).astype(BF16)
        vtq_c = np.ascontiguousarray(vt_c[:, 0:IPC].astype(np.float32))
        # per-row L1 constant: W1@v_i + b1, stacked per pair into one column
        c2 = w1bf @ vtq_c + b1[:, None]                              # (64, IPC)
        cbias_c = np.empty((128, IPC // 2), np.float32)
        cbias_c[0:H1] = c2[:, 0::2]
        cbias_c[H1:128] = c2[:, 1::2]
        in_maps.append(
            {
                "vt": vt_c,
                "vtq": vtq_c,
                "w1t2": w1t2,
                "m1t": m1t,
                "w2bd": w2bd,
                "w3v": w3v,
                "cbias": cbias_c,
                "b2s": b2st,
                "idm": idm,
            }
        )
    return in_maps


def core_geometry(c):
    """Per-core row ownership and column permutation.

    Each core's first 128 rows (batch A) compute all 512 columns; its second
    128 rows (batch B) skip local columns 0:128 — which are exactly batch A's
    rows, so the missing logit block is recovered on-device by transposing
    a[A-rows, B-rows] (the pairwise MLP is symmetric in (i, j))."""
    b, half = c // 2, c % 2
    if half == 0:
        perm = np.arange(N)
        rowmap = np.arange(0, IPC)
    else:
        perm = np.concatenate(
            [np.arange(384, 512), np.arange(256, 384), np.arange(0, 256)]
        )
        rowmap = perm[0:IPC]
    return b, rowmap, perm


_NC_CACHE = [None]


def get_nc():
    if _NC_CACHE[0] is None:
        nc = build_nc()
        nc.finalize()  # runs the Bacc lowering passes (reg alloc, sem split)
        _NC_CACHE[0] = nc
    return _NC_CACHE[0]


def run(inputs, trace=False, **kw):
    nc = get_nc()
    in_maps = make_in_maps(
        inputs["v"], inputs["W1"], inputs["b1"], inputs["W2"], inputs["b2"],
        inputs["W3"],
    )
    res = run_bass_kernel_spmd(nc, in_maps, list(range(NCORES)), trace=trace, **kw)
    out = np.empty((B, N, N), np.float32)
    for c in range(NCORES):
        b, rowmap, perm = core_geometry(c)
        out[b][np.ix_(rowmap, perm)] = np.asarray(
            res.results[c]["out"], np.float32
        )
    return out, res


def kernel(**inputs):
    out, _ = run(inputs, trace=False)
    return out
